# revision 31
# baseline (speedup 1.0000x reference)
"""Causal self-attention (B=2, T=2048, D=1024, H=16) on 8 trn2 NeuronCores.

Sharding: core = (batch b, head-group g) with 4 heads per group.
Each core computes its heads' full attention plus its slice of the output
projection; the host sums the 4 per-group partial outputs per batch.

Layout choice: scores are computed transposed ([s, t], keys on partitions)
so softmax's sum over s comes for free from an extra all-ones column in the
attn@v stationary operand, and the attention output lands pre-transposed
([head_dim, t]) which is exactly the lhsT layout the output projection needs.
"""

import numpy as np
from contextlib import ExitStack

import concourse.bass as bass
import concourse.tile as tile
from concourse import mybir
from concourse.bass_utils import run_bass_kernel_spmd
from concourse.vector_clock import ScopedClock, VectorClock

B, T, D, H = 2, 2048, 1024, 16
HD = D // H            # 64
HG = 4                 # heads per core
GD = HG * HD           # 256, per-core projection width
NCk = D // 128         # 8 contraction chunks over D
NS = T // 128          # 16 s-tiles
TCH = 512              # t-chunk width
NJ = T // TCH          # 4 t-chunks
F32 = mybir.dt.float32
F32R = mybir.dt.float32r  # TF32-class matmul inputs: 4x PE throughput vs fp32

# ---------------------------------------------------------------------------
# Walrus on this image accepts only 1 sync-wait slot on regular instructions
# (2 on EventSemaphore), but Tile emits multi-wait instructions. Split excess
# waits onto EventSemaphore instructions inserted before, same engine.


def _drain_and_barrier_split(self, tick_clock, wait_clock):
    vc = tick_clock.global_clock
    n = len(vc)
    procs = [(p, vc[p]) for p in range(n) if vc[p] > 0]
    for k in range(len(procs)):
        vec = [0] * n
        p, t = procs[k]
        vec[p] = t
        d = self.nc.sync.drain()
        wait_clock.add_sem_waits(d.ins, ScopedClock({None: VectorClock(vec)}))
    self.nc.all_engine_barrier()
    assert self.sems is not None
    popped = self.nc._tile_sem_poison_stack.pop()
    assert popped is self._sem_poison
    self.nc.clear_and_free_semaphores(list(self.sems.allocated().values()))
    self.nc.all_engine_barrier()


def _split_waits(ordered):
    for bb_name, insts in ordered.items():
        out = []
        for inst in insts:
            si = inst.sync_info
            waits = list(si.on_wait) if si is not None and si.on_wait else []
            if len(waits) > 1:
                extra, keep = waits[:-1], waits[-1:]
                for k in range(0, len(extra), 2):
                    ev = mybir.InstEventSemaphore(
                        name=f"{inst.name}-sw{k}", ins=[], outs=[]
                    )
                    ev.engine = inst.engine
                    ev.debug = inst.debug
                    ev.sync_info = mybir.SyncInfo(
                        on_update=[], on_wait=extra[k : k + 2]
                    )
                    out.append(ev)
                inst.sync_info = mybir.SyncInfo(
                    on_update=list(si.on_update) if si.on_update else [],
                    on_wait=keep,
                )
            out.append(inst)
        ordered[bb_name] = out


_patched = False


def _apply_patches():
    global _patched
    if _patched:
        return
    _patched = True
    tile.TileContext._drain_and_barrier = _drain_and_barrier_split
    orig_lower = tile.TileContext._lower_ordered_insts

    def lower_with_split(self, ordered):
        _split_waits(ordered)
        return orig_lower(self, ordered)

    tile.TileContext._lower_ordered_insts = lower_with_split


# ---------------------------------------------------------------------------


def _build_nc(reps=1):
    nc = bass.Bass(trn_type="TRN2", debug=False)
    xT = nc.dram_tensor("xT", [D, T], F32R, kind="ExternalInput").ap()
    wq = nc.dram_tensor("wq", [D, GD], F32R, kind="ExternalInput").ap()
    wk = nc.dram_tensor("wk", [D, GD], F32R, kind="ExternalInput").ap()
    wv = nc.dram_tensor("wv", [D, GD], F32R, kind="ExternalInput").ap()
    wo = nc.dram_tensor("wo", [GD, D], F32R, kind="ExternalInput").ap()
    vone = nc.dram_tensor("vone", [128, HD], F32R, kind="ExternalInput").ap()
    y = nc.dram_tensor("y", [T, D], F32, kind="ExternalOutput").ap()

    xT_d = xT.rearrange("(n p) t -> n p t", p=128)     # [8, 128, 2048]
    wq_d = wq.rearrange("(n p) d -> n p d", p=128)     # [8, 128, 256]
    wk_d = wk.rearrange("(n p) d -> n p d", p=128)
    wv_d = wv.rearrange("(n p) d -> n p d", p=128)
    wo_d = wo.rearrange("(n p) d -> n p d", p=128)     # [2, 128, 1024]
    y_d = y.rearrange("(n p) d -> n p d", p=128)       # [16, 128, 1024]

    with ExitStack() as outer:
        tc = outer.enter_context(tile.TileContext(nc))
        for _rep in range(reps):
            _one_rep(nc, tc, xT_d, wq_d, wk_d, wv_d, wo_d, y_d, vone)
    return nc


def _one_rep(nc, tc, xT_d, wq_d, wk_d, wv_d, wo_d, y_d, vone):
    with ExitStack() as top:
        qkv = top.enter_context(tc.tile_pool(name="qkv", bufs=1))
        ptp = top.enter_context(tc.tile_pool(name="ptp", bufs=5))
        nrm = top.enter_context(tc.tile_pool(name="nrm", bufs=3))
        yout = top.enter_context(tc.tile_pool(name="yout", bufs=3))

        # persistent sbuf tensors
        qT2 = [qkv.tile([128, T], F32R, tag=f"qT{m}", name=f"qT{m}") for m in range(2)]
        kT2 = [qkv.tile([128, T], F32R, tag=f"kT{m}", name=f"kT{m}") for m in range(2)]
        aoT = [qkv.tile([128, T], F32R, tag=f"aoT{m}", name=f"aoT{m}") for m in range(2)]
        vext = [
            qkv.tile([128, HG * (HD + 1)], F32R, tag=f"v{i}", name=f"v{i}")
            for i in range(NS)
        ]
        wo_sb = [
            qkv.tile([128, D], F32R, tag=f"wo{m}", name=f"wo{m}") for m in range(2)
        ]
        ones_sb = qkv.tile([128, HD], F32R, tag="ones", name="ones_sb")

        # ---------------- stage 1: projections ----------------
        with ExitStack() as s1:
            xw = s1.enter_context(tc.tile_pool(name="xw", bufs=1))
            ps1 = s1.enter_context(tc.tile_pool(name="ps1", bufs=2, space="PSUM"))

            xT_sb = [
                xw.tile([128, T], F32R, tag=f"xT{c}", name=f"xT{c}") for c in range(NCk)
            ]
            wq_sb = [
                xw.tile([128, GD], F32R, tag=f"wq{c}", name=f"wq{c}")
                for c in range(NCk)
            ]
            wk_sb = [
                xw.tile([128, GD], F32R, tag=f"wk{c}", name=f"wk{c}")
                for c in range(NCk)
            ]
            wv_sb = [
                xw.tile([128, GD], F32R, tag=f"wv{c}", name=f"wv{c}")
                for c in range(NCk)
            ]
            # x and the q-weights gate the first matmul group: load them first
            for c in range(NCk):
                nc.sync.dma_start(out=xT_sb[c][:], in_=xT_d[c])
            for c in range(NCk):
                nc.sync.dma_start(out=wq_sb[c][:], in_=wq_d[c])
            for c in range(NCk):
                nc.sync.dma_start(out=wk_sb[c][:], in_=wk_d[c])
            for c in range(NCk):
                nc.sync.dma_start(out=wv_sb[c][:], in_=wv_d[c])
            nc.sync.dma_start(out=ones_sb[:], in_=vone)
            for m in range(2):
                nc.sync.dma_start(out=wo_sb[m][:], in_=wo_d[m])

            # qT, kT: [dh-pair tile m][:, t]  (partitions = 2 heads x 64)
            for dst, w_sb in ((qT2, wq_sb), (kT2, wk_sb)):
                for m in range(2):
                    for j in range(NJ):
                        acc = ps1.tile([128, TCH], F32, tag="ps1a", name="acc")
                        for c in range(NCk):
                            nc.tensor.matmul(
                                acc[:],
                                w_sb[c][:, m * 128 : (m + 1) * 128],
                                xT_sb[c][:, j * TCH : (j + 1) * TCH],
                                start=(c == 0),
                                stop=(c == NCk - 1),
                            )
                        nc.vector.tensor_copy(
                            dst[m][:, j * TCH : (j + 1) * TCH], acc[:]
                        )

            # v: per s-tile [128, 4*(64+1)] with all-ones denominator column
            for i in range(NS):
                acc = ps1.tile([128, GD], F32, tag="ps1v", name="accv")
                for c in range(NCk):
                    nc.tensor.matmul(
                        acc[:],
                        xT_sb[c][:, i * 128 : (i + 1) * 128],
                        wv_sb[c][:],
                        start=(c == 0),
                        stop=(c == NCk - 1),
                    )
                v_view = vext[i].rearrange("p (h e) -> p h e", e=HD + 1)
                nc.vector.tensor_copy(
                    v_view[:, :, 0:HD], acc.rearrange("p (h e) -> p h e", e=HD)
                )
                nc.vector.tensor_copy(
                    v_view[:, :, HD : HD + 1],
                    ones_sb[:, 0:HG].rearrange("p (h o) -> p h o", o=1),
                )

        # ---------------- stage 2 + 3: attention, fused out-proj ----------
        # j outer so both head-pairs finish aoT[:, chunk j] together and the
        # output projection for that chunk interleaves with the next chunk's
        # softmax work on the other engines.
        s2 = top.enter_context(ExitStack())
        pso = s2.enter_context(tc.tile_pool(name="pso", bufs=3, space="PSUM"))
        psv = s2.enter_context(tc.tile_pool(name="psv", bufs=3, space="PSUM"))
        psb = s2.enter_context(tc.tile_pool(name="psb", bufs=1, space="PSUM"))
        ps3 = s2.enter_context(tc.tile_pool(name="ps3", bufs=1, space="PSUM"))
        for j in range(NJ):
            n_i = 4 * j + 4
            for p in range(2):  # head pair -> local heads (2p, 2p+1)
                outp = [
                    psv.tile([HD + 1, TCH], F32, tag="outp", name=f"outp{hp}")
                    for hp in range(2)
                ]
                for i in range(n_i):
                    # causally-valid columns of this (i, j) block start at c0;
                    # keep matmul N >= 256 so fp32r stays at 1 cycle/row
                    c0 = max(0, 128 * i - TCH * j)
                    c0n = min(c0, TCH - 256)
                    for hp in range(2):
                        sc = pso.tile([128, TCH], F32, tag="sc", name="sc")
                        pt = ptp.tile([128, TCH], F32R, tag="pt", name="pt")
                        nc.tensor.matmul(
                            sc[:, c0n:TCH],
                            kT2[p][hp * 64 : hp * 64 + 64, i * 128 : (i + 1) * 128],
                            qT2[p][hp * 64 : hp * 64 + 64, j * TCH + c0n : (j + 1) * TCH],
                            start=True,
                            stop=True,
                        )
                        nc.scalar.activation(
                            pt[:, c0n:TCH],
                            sc[:, c0n:TCH],
                            mybir.ActivationFunctionType.Exp,
                            scale=1.0 / np.sqrt(HD),
                        )
                        if i // 4 == j:
                            me = min(c0 + 128, TCH)
                            nc.gpsimd.affine_select(
                                out=pt[:, c0n:me],
                                in_=pt[:, c0n:me],
                                compare_op=mybir.AluOpType.is_ge,
                                fill=0.0,
                                base=j * TCH + c0n - i * 128,
                                pattern=[[1, me - c0n]],
                                channel_multiplier=-1,
                            )
                        hl = 2 * p + hp
                        nc.tensor.matmul(
                            outp[hp][:, c0n:TCH],
                            vext[i][:, hl * (HD + 1) : (hl + 1) * (HD + 1)],
                            pt[:, c0n:TCH],
                            start=(i == 0),
                            stop=(i == n_i - 1),
                        )
                for hp in range(2):
                    # 1/d via exp(-ln(d)) on ScalarE (vector.reciprocal is
                    # 8 cyc/elem and the custom-DVE approx doesn't compile
                    # on this walrus build)
                    lnd = nrm.tile([1, TCH], F32, tag="lnd", name="lnd")
                    nc.scalar.activation(
                        lnd[:],
                        outp[hp][HD : HD + 1, :],
                        mybir.ActivationFunctionType.Ln,
                    )
                    recip = nrm.tile([1, TCH], F32R, tag="recip", name="recip")
                    nc.scalar.activation(
                        recip[:],
                        lnd[:],
                        mybir.ActivationFunctionType.Exp,
                        scale=-1.0,
                    )
                    # broadcast 1/d across the 64 head-dim partitions via PE
                    bcp = psb.tile([HD, TCH], F32, tag="bcp", name="bcp")
                    nc.tensor.matmul(
                        bcp[:], ones_sb[0:1, :], recip[:], start=True, stop=True
                    )
                    bcast = nrm.tile([HD, TCH], F32, tag="bcast", name="bcast")
                    nc.vector.tensor_copy(bcast[:], bcp[:])
                    nc.vector.tensor_mul(
                        aoT[p][hp * 64 : hp * 64 + 64, j * TCH : (j + 1) * TCH],
                        outp[hp][0:HD, :],
                        bcast[:],
                    )
            # out-projection for this chunk's four 128-row t-tiles
            for tt in range(4 * j, 4 * j + 4):
                y_sb = yout.tile([128, D], F32, tag="ysb", name="y_sb")
                for e in range(2):
                    acc = ps3.tile([128, TCH], F32, tag="ps3", name="acc3")
                    for m in range(2):
                        nc.tensor.matmul(
                            acc[:],
                            aoT[m][:, tt * 128 : (tt + 1) * 128],
                            wo_sb[m][:, e * TCH : (e + 1) * TCH],
                            start=(m == 0),
                            stop=(m == 1),
                        )
                    nc.vector.tensor_copy(
                        y_sb[:, e * TCH : (e + 1) * TCH], acc[:]
                    )
                nc.gpsimd.dma_start(out=y_d[tt][:], in_=y_sb[:])


_nc_cache = None


def _get_nc():
    global _nc_cache
    if _nc_cache is None:
        _apply_patches()
        _nc_cache = _build_nc()
    return _nc_cache


def kernel(x, Wq, Wk, Wv, Wo, mask, _want_results=False, _trace=False):
    x = np.asarray(x, dtype=np.float32)
    Wq = np.asarray(Wq, dtype=np.float32)
    Wk = np.asarray(Wk, dtype=np.float32)
    Wv = np.asarray(Wv, dtype=np.float32)
    Wo = np.asarray(Wo, dtype=np.float32)

    nc = _get_nc()
    in_maps = []
    for core in range(8):
        b, g = divmod(core, HG)
        sl = slice(g * GD, (g + 1) * GD)
        in_maps.append(
            {
                "xT": np.ascontiguousarray(x[b].T),
                "wq": np.ascontiguousarray(Wq[sl, :].T),
                "wk": np.ascontiguousarray(Wk[sl, :].T),
                "wv": np.ascontiguousarray(Wv[sl, :].T),
                "wo": np.ascontiguousarray(Wo[:, sl].T),
                "vone": np.ones((128, HD), np.float32),
            }
        )
    res = run_bass_kernel_spmd(
        nc, in_maps, core_ids=list(range(8)), trace=_trace
    )
    y = np.zeros((B, T, D), dtype=np.float32)
    for core in range(8):
        b = core // HG
        y[b] += res.results[core]["y"]
    if _want_results:
        return y, res
    return y


# revision 33
# speedup vs baseline: 4.7142x; 4.7142x over previous
"""Causal self-attention (B=2, T=2048, D=1024, H=16) on 8 trn2 NeuronCores.

Sharding: core = (batch b, head-group g) with 4 heads per group.
Each core computes its heads' full attention plus its slice of the output
projection; the host sums the 4 per-group partial outputs per batch.

Layout choice: scores are computed transposed ([s, t], keys on partitions)
so softmax's sum over s comes for free from an extra all-ones column in the
attn@v stationary operand, and the attention output lands pre-transposed
([head_dim, t]) which is exactly the lhsT layout the output projection needs.
"""

import numpy as np
from contextlib import ExitStack

import concourse.bass as bass
import concourse.tile as tile
from concourse import mybir
from concourse.bass_utils import run_bass_kernel_spmd
from concourse.vector_clock import ScopedClock, VectorClock

B, T, D, H = 2, 2048, 1024, 16
HD = D // H            # 64
HG = 4                 # heads per core
GD = HG * HD           # 256, per-core projection width
NCk = D // 128         # 8 contraction chunks over D
NS = T // 128          # 16 s-tiles
TCH = 512              # t-chunk width
NJ = T // TCH          # 4 t-chunks
F32 = mybir.dt.float32
F32R = mybir.dt.float32r  # TF32-class matmul inputs: 4x PE throughput vs fp32

# ---------------------------------------------------------------------------
# Walrus on this image accepts only 1 sync-wait slot on regular instructions
# (2 on EventSemaphore), but Tile emits multi-wait instructions. Split excess
# waits onto EventSemaphore instructions inserted before, same engine.


def _drain_and_barrier_split(self, tick_clock, wait_clock):
    vc = tick_clock.global_clock
    n = len(vc)
    procs = [(p, vc[p]) for p in range(n) if vc[p] > 0]
    for k in range(len(procs)):
        vec = [0] * n
        p, t = procs[k]
        vec[p] = t
        d = self.nc.sync.drain()
        wait_clock.add_sem_waits(d.ins, ScopedClock({None: VectorClock(vec)}))
    self.nc.all_engine_barrier()
    assert self.sems is not None
    popped = self.nc._tile_sem_poison_stack.pop()
    assert popped is self._sem_poison
    self.nc.clear_and_free_semaphores(list(self.sems.allocated().values()))
    self.nc.all_engine_barrier()


def _split_waits(ordered):
    for bb_name, insts in ordered.items():
        out = []
        for inst in insts:
            si = inst.sync_info
            waits = list(si.on_wait) if si is not None and si.on_wait else []
            if len(waits) > 1:
                extra, keep = waits[:-1], waits[-1:]
                for k in range(0, len(extra), 2):
                    ev = mybir.InstEventSemaphore(
                        name=f"{inst.name}-sw{k}", ins=[], outs=[]
                    )
                    ev.engine = inst.engine
                    ev.debug = inst.debug
                    ev.sync_info = mybir.SyncInfo(
                        on_update=[], on_wait=extra[k : k + 2]
                    )
                    out.append(ev)
                inst.sync_info = mybir.SyncInfo(
                    on_update=list(si.on_update) if si.on_update else [],
                    on_wait=keep,
                )
            out.append(inst)
        ordered[bb_name] = out


_patched = False


def _apply_patches():
    global _patched
    if _patched:
        return
    _patched = True
    tile.TileContext._drain_and_barrier = _drain_and_barrier_split
    orig_lower = tile.TileContext._lower_ordered_insts

    def lower_with_split(self, ordered):
        _split_waits(ordered)
        return orig_lower(self, ordered)

    tile.TileContext._lower_ordered_insts = lower_with_split


# ---------------------------------------------------------------------------


def _build_nc(reps=1):
    nc = bass.Bass(trn_type="TRN2", debug=False)
    xT = nc.dram_tensor("xT", [D, T], F32R, kind="ExternalInput").ap()
    wq = nc.dram_tensor("wq", [D, GD], F32R, kind="ExternalInput").ap()
    wk = nc.dram_tensor("wk", [D, GD], F32R, kind="ExternalInput").ap()
    wv = nc.dram_tensor("wv", [D, GD], F32R, kind="ExternalInput").ap()
    wo = nc.dram_tensor("wo", [GD, D], F32R, kind="ExternalInput").ap()
    vone = nc.dram_tensor("vone", [128, HD], F32R, kind="ExternalInput").ap()
    y = nc.dram_tensor("y", [T, D], F32, kind="ExternalOutput").ap()

    xT_d = xT.rearrange("(n p) t -> n p t", p=128)     # [8, 128, 2048]
    wq_d = wq.rearrange("(n p) d -> n p d", p=128)     # [8, 128, 256]
    wk_d = wk.rearrange("(n p) d -> n p d", p=128)
    wv_d = wv.rearrange("(n p) d -> n p d", p=128)
    wo_d = wo.rearrange("(n p) d -> n p d", p=128)     # [2, 128, 1024]
    y_d = y.rearrange("(n p) d -> n p d", p=128)       # [16, 128, 1024]

    with ExitStack() as outer:
        tc = outer.enter_context(tile.TileContext(nc))
        for _rep in range(reps):
            _one_rep(nc, tc, xT_d, wq_d, wk_d, wv_d, wo_d, y_d, vone)
    return nc


def _one_rep(nc, tc, xT_d, wq_d, wk_d, wv_d, wo_d, y_d, vone):
    with ExitStack() as top:
        qkv = top.enter_context(tc.tile_pool(name="qkv", bufs=1))
        ptp = top.enter_context(tc.tile_pool(name="ptp", bufs=5))
        nrm = top.enter_context(tc.tile_pool(name="nrm", bufs=3))
        yout = top.enter_context(tc.tile_pool(name="yout", bufs=3))

        # persistent sbuf tensors
        qT2 = [qkv.tile([128, T], F32R, tag=f"qT{m}", name=f"qT{m}") for m in range(2)]
        kT2 = [qkv.tile([128, T], F32R, tag=f"kT{m}", name=f"kT{m}") for m in range(2)]
        aoT = [qkv.tile([128, T], F32R, tag=f"aoT{m}", name=f"aoT{m}") for m in range(2)]
        vext = [
            qkv.tile([128, HG * (HD + 1)], F32R, tag=f"v{i}", name=f"v{i}")
            for i in range(NS)
        ]
        wo_sb = [
            qkv.tile([128, D], F32R, tag=f"wo{m}", name=f"wo{m}") for m in range(2)
        ]
        ones_sb = qkv.tile([128, HD], F32R, tag="ones", name="ones_sb")

        # ---------------- stage 1: projections ----------------
        with ExitStack() as s1:
            xw = s1.enter_context(tc.tile_pool(name="xw", bufs=1))
            ps1 = s1.enter_context(tc.tile_pool(name="ps1", bufs=2, space="PSUM"))

            xT_sb = [
                xw.tile([128, T], F32R, tag=f"xT{c}", name=f"xT{c}") for c in range(NCk)
            ]
            wq_sb = [
                xw.tile([128, GD], F32R, tag=f"wq{c}", name=f"wq{c}")
                for c in range(NCk)
            ]
            wk_sb = [
                xw.tile([128, GD], F32R, tag=f"wk{c}", name=f"wk{c}")
                for c in range(NCk)
            ]
            wv_sb = [
                xw.tile([128, GD], F32R, tag=f"wv{c}", name=f"wv{c}")
                for c in range(NCk)
            ]
            # x and the q-weights gate the first matmul group: load them first
            for c in range(NCk):
                nc.sync.dma_start(out=xT_sb[c][:], in_=xT_d[c])
            for c in range(NCk):
                nc.sync.dma_start(out=wq_sb[c][:], in_=wq_d[c])
            for c in range(NCk):
                nc.sync.dma_start(out=wk_sb[c][:], in_=wk_d[c])
            for c in range(NCk):
                nc.sync.dma_start(out=wv_sb[c][:], in_=wv_d[c])
            nc.sync.dma_start(out=ones_sb[:], in_=vone)
            for m in range(2):
                nc.sync.dma_start(out=wo_sb[m][:], in_=wo_d[m])

            # qT, kT: [dh-pair tile m][:, t]  (partitions = 2 heads x 64)
            for dst, w_sb in ((qT2, wq_sb), (kT2, wk_sb)):
                for m in range(2):
                    for j in range(NJ):
                        acc = ps1.tile([128, TCH], F32, tag="ps1a", name="acc")
                        for c in range(NCk):
                            nc.tensor.matmul(
                                acc[:],
                                w_sb[c][:, m * 128 : (m + 1) * 128],
                                xT_sb[c][:, j * TCH : (j + 1) * TCH],
                                start=(c == 0),
                                stop=(c == NCk - 1),
                            )
                        nc.vector.tensor_copy(
                            dst[m][:, j * TCH : (j + 1) * TCH], acc[:]
                        )

            # v: per s-tile [128, 4*(64+1)] with all-ones denominator column
            for i in range(NS):
                acc = ps1.tile([128, GD], F32, tag="ps1v", name="accv")
                for c in range(NCk):
                    nc.tensor.matmul(
                        acc[:],
                        xT_sb[c][:, i * 128 : (i + 1) * 128],
                        wv_sb[c][:],
                        start=(c == 0),
                        stop=(c == NCk - 1),
                    )
                v_view = vext[i].rearrange("p (h e) -> p h e", e=HD + 1)
                nc.vector.tensor_copy(
                    v_view[:, :, 0:HD], acc.rearrange("p (h e) -> p h e", e=HD)
                )
                nc.vector.tensor_copy(
                    v_view[:, :, HD : HD + 1],
                    ones_sb[:, 0:HG].rearrange("p (h o) -> p h o", o=1),
                )

        # ---------------- stage 2 + 3: attention, fused out-proj ----------
        # j outer so both head-pairs finish aoT[:, chunk j] together and the
        # output projection for that chunk interleaves with the next chunk's
        # softmax work on the other engines.
        s2 = top.enter_context(ExitStack())
        pso = s2.enter_context(tc.tile_pool(name="pso", bufs=3, space="PSUM"))
        psv = s2.enter_context(tc.tile_pool(name="psv", bufs=3, space="PSUM"))
        psb = s2.enter_context(tc.tile_pool(name="psb", bufs=1, space="PSUM"))
        ps3 = s2.enter_context(tc.tile_pool(name="ps3", bufs=1, space="PSUM"))
        for j in range(NJ):
            n_i = 4 * j + 4
            for p in range(2):  # head pair -> local heads (2p, 2p+1)
                outp = [
                    psv.tile([HD + 1, TCH], F32, tag="outp", name=f"outp{hp}")
                    for hp in range(2)
                ]
                for i in range(n_i):
                    # causally-valid columns of this (i, j) block start at c0;
                    # keep matmul N >= 256 so fp32r stays at 1 cycle/row
                    c0 = max(0, 128 * i - TCH * j)
                    c0n = min(c0, TCH - 256)
                    for hp in range(2):
                        sc = pso.tile([128, TCH], F32, tag="sc", name="sc")
                        pt = ptp.tile([128, TCH], F32R, tag="pt", name="pt")
                        nc.tensor.matmul(
                            sc[:, c0n:TCH],
                            kT2[p][hp * 64 : hp * 64 + 64, i * 128 : (i + 1) * 128],
                            qT2[p][hp * 64 : hp * 64 + 64, j * TCH + c0n : (j + 1) * TCH],
                            start=True,
                            stop=True,
                        )
                        nc.scalar.activation(
                            pt[:, c0n:TCH],
                            sc[:, c0n:TCH],
                            mybir.ActivationFunctionType.Exp,
                            scale=1.0 / np.sqrt(HD),
                        )
                        if i // 4 == j:
                            me = min(c0 + 128, TCH)
                            nc.gpsimd.affine_select(
                                out=pt[:, c0n:me],
                                in_=pt[:, c0n:me],
                                compare_op=mybir.AluOpType.is_ge,
                                fill=0.0,
                                base=j * TCH + c0n - i * 128,
                                pattern=[[1, me - c0n]],
                                channel_multiplier=-1,
                            )
                        hl = 2 * p + hp
                        nc.tensor.matmul(
                            outp[hp][:, c0n:TCH],
                            vext[i][:, hl * (HD + 1) : (hl + 1) * (HD + 1)],
                            pt[:, c0n:TCH],
                            start=(i == 0),
                            stop=(i == n_i - 1),
                        )
                for hp in range(2):
                    # 1/d via exp(-ln(d)) on ScalarE (vector.reciprocal is
                    # 8 cyc/elem and the custom-DVE approx doesn't compile
                    # on this walrus build)
                    lnd = nrm.tile([1, TCH], F32, tag="lnd", name="lnd")
                    nc.scalar.activation(
                        lnd[:],
                        outp[hp][HD : HD + 1, :],
                        mybir.ActivationFunctionType.Ln,
                    )
                    recip = nrm.tile([1, TCH], F32R, tag="recip", name="recip")
                    nc.scalar.activation(
                        recip[:],
                        lnd[:],
                        mybir.ActivationFunctionType.Exp,
                        scale=-1.0,
                    )
                    # broadcast 1/d across the 64 head-dim partitions via PE
                    bcp = psb.tile([HD, TCH], F32, tag="bcp", name="bcp")
                    nc.tensor.matmul(
                        bcp[:], ones_sb[0:1, :], recip[:], start=True, stop=True
                    )
                    bcast = nrm.tile([HD, TCH], F32, tag="bcast", name="bcast")
                    nc.vector.tensor_copy(bcast[:], bcp[:])
                    nc.vector.tensor_mul(
                        aoT[p][hp * 64 : hp * 64 + 64, j * TCH : (j + 1) * TCH],
                        outp[hp][0:HD, :],
                        bcast[:],
                    )
            # out-projection for this chunk's four 128-row t-tiles
            for tt in range(4 * j, 4 * j + 4):
                y_sb = yout.tile([128, D], F32, tag="ysb", name="y_sb")
                for e in range(2):
                    acc = ps3.tile([128, TCH], F32, tag="ps3", name="acc3")
                    for m in range(2):
                        nc.tensor.matmul(
                            acc[:],
                            aoT[m][:, tt * 128 : (tt + 1) * 128],
                            wo_sb[m][:, e * TCH : (e + 1) * TCH],
                            start=(m == 0),
                            stop=(m == 1),
                        )
                    nc.vector.tensor_copy(
                        y_sb[:, e * TCH : (e + 1) * TCH], acc[:]
                    )
                nc.gpsimd.dma_start(out=y_d[tt][:], in_=y_sb[:])


_nc_cache = None


def _get_nc():
    global _nc_cache
    if _nc_cache is None:
        _apply_patches()
        _nc_cache = _build_nc()
    return _nc_cache


def kernel(x, Wq, Wk, Wv, Wo, mask, _want_results=False, _trace=False):
    x = np.asarray(x, dtype=np.float32)
    Wq = np.asarray(Wq, dtype=np.float32)
    Wk = np.asarray(Wk, dtype=np.float32)
    Wv = np.asarray(Wv, dtype=np.float32)
    Wo = np.asarray(Wo, dtype=np.float32)

    nc = _get_nc()
    in_maps = []
    for core in range(8):
        b, g = divmod(core, HG)
        sl = slice(g * GD, (g + 1) * GD)
        in_maps.append(
            {
                "xT": np.ascontiguousarray(x[b].T),
                "wq": np.ascontiguousarray(Wq[sl, :].T),
                "wk": np.ascontiguousarray(Wk[sl, :].T),
                "wv": np.ascontiguousarray(Wv[sl, :].T),
                "wo": np.ascontiguousarray(Wo[:, sl].T),
                "vone": np.ones((128, HD), np.float32),
            }
        )
    res = run_bass_kernel_spmd(
        nc, in_maps, core_ids=list(range(8)), trace=_trace
    )
    y = np.zeros((B, T, D), dtype=np.float32)
    for core in range(8):
        b = core // HG
        y[b] += res.results[core]["y"]
    if _want_results:
        return y, res
    return y
